# revision 11
# baseline (speedup 1.0000x reference)
"""Trainium2 Bass kernel for nn_FCGF_MLP3 (MLP -> BN -> relu x2 -> segment mean -> L2 norm).

Contract: kernel(**inputs) takes FULL unsharded numpy inputs (as produced by
setup_inputs) and returns the FULL [64, 256] float32 output.  Internally the
points are sharded across 8 NeuronCores (whole segments per core); BN batch
stats are combined with two tiny on-device AllReduces.

Per-core dataflow (npts = 65536 points, channels on partitions):
  phase 1: stream pre-transposed x (bf16), block-diag W1 matmul computes two
           512-pt chunks per matmul -> h1raw [128, npts/2] bf16 resident in
           SBUF; ACT fuses the PSUM->SBUF copy with a per-channel running
           sum; DVE fuses square+sum.  (b1/b2 cancel through BN and are
           dropped entirely.)
  AR1:     AllReduce [64,2] of (sum, sumsq) -> BN1 affine (a1, c1) on device.
  phase 2: ACT in-place relu(a1*h+c1) with free running sum (s1); PE
           transposes 64x128 chunks; PE Gram G1 = h1^T h1 in PSUM.
  AR2:     AllReduce [64,65] of (G1 | s1) -> BN2 stats analytically:
           var2 = q/n - (r/n)^2 with q = diag(W2 G1 W2^T), r = W2 s1.
           a2 folds into W2 columns, c2 stays as per-partition bias.
  phase 3: mm2 (bf16) -> PSUM; fused relu+segment-sum split between ACT
           (activation accum_out) and DVE (scalar_tensor_tensor accum_out);
           segment means; L2 norm via PE column-sum + sqrt + reciprocal.
"""

import contextlib
import functools

import numpy as np
import ml_dtypes

import concourse.bass as bass
import concourse.bacc as bacc
import concourse.tile as tile
from concourse import mybir
from concourse.bass_utils import run_bass_kernel_spmd

BF16 = mybir.dt.bfloat16
F32 = mybir.dt.float32
AF = mybir.ActivationFunctionType
ALU = mybir.AluOpType

N_CORES = 8
N_SEG = 64
SEG_PER_CORE = N_SEG // N_CORES  # 8
BN_EPS = 1e-5
L2_EPS = 1e-12

bf16 = ml_dtypes.bfloat16


# --------------------------------------------------------------------------
# device program
# --------------------------------------------------------------------------

def build_program(npts, n_total, stage=10):
    """Build the per-core bass program.

    Layout: point p of the core lives in column (p mod C) of partition-half
    (p div C), C = npts/2; h1[ch + 64*half, col].  Segments 0..3 of the core
    are in half 0, segments 4..7 in half 1.

    stage < 10 truncates the program after a phase (debug bisection).
    """
    assert npts % (8 * 512) == 0
    C = npts // 2               # columns per half
    seg_cols = npts // 8        # one segment's column span (within one half)
    GSZ = 2048 if C % 2048 == 0 else 512  # columns per PSUM group
    NG = C // GSZ               # groups per half
    n_chunk_t = npts // 128     # number of 128-point transpose chunks

    nc = bacc.Bacc(
        "TRN2",
        target_bir_lowering=False,
        debug=False,
        enable_asserts=True,
        num_devices=N_CORES,
    )

    # ---- I/O ----
    xp_d = nc.dram_tensor("xp", [64, C], BF16, kind="ExternalInput")
    w1bd_d = nc.dram_tensor("w1bd", [64, 128], BF16, kind="ExternalInput")
    g1r_d = nc.dram_tensor("g1r", [128, 1], F32, kind="ExternalInput")
    be1r_d = nc.dram_tensor("be1r", [128, 1], F32, kind="ExternalInput")
    w2t_d = nc.dram_tensor("w2t", [128, 256], F32, kind="ExternalInput")
    g2p_d = nc.dram_tensor("g2p", [128, 2], F32, kind="ExternalInput")
    be2p_d = nc.dram_tensor("be2p", [128, 2], F32, kind="ExternalInput")
    id64_d = nc.dram_tensor("id64", [128, 64], BF16, kind="ExternalInput")
    ones_d = nc.dram_tensor("ones128", [128, 1], F32, kind="ExternalInput")
    out_d = nc.dram_tensor("out", [SEG_PER_CORE, 256], F32, kind="ExternalOutput")

    inv_n = 1.0 / float(n_total)
    inv_seg = 1.0 / float(npts // 8)

    def _emit(tc, ctx):
        singles = ctx.enter_context(tc.tile_pool(name="singles", bufs=1))
        persist = ctx.enter_context(tc.tile_pool(name="persist", bufs=1))
        stats = ctx.enter_context(tc.tile_pool(name="stats", bufs=1))
        dram = ctx.enter_context(tc.tile_pool(name="dram", bufs=1, space="DRAM"))

        def dummy_out():
            dummy = stats.tile([SEG_PER_CORE, 256], F32, name="dummy")
            nc.vector.memset(dummy, 1.0)
            nc.sync.dma_start(out=out_d[:, :], in_=dummy)

        # ---- small constants into SBUF ----
        w1bd = singles.tile([64, 128], BF16)
        nc.sync.dma_start(out=w1bd, in_=w1bd_d[:, :])
        g1r = singles.tile([128, 1], F32)
        nc.sync.dma_start(out=g1r, in_=g1r_d[:, :])
        be1r = singles.tile([128, 1], F32)
        nc.sync.dma_start(out=be1r, in_=be1r_d[:, :])
        w2t = singles.tile([128, 256], F32)
        nc.sync.dma_start(out=w2t, in_=w2t_d[:, :])
        g2p = singles.tile([128, 2], F32)
        nc.sync.dma_start(out=g2p, in_=g2p_d[:, :])
        be2p = singles.tile([128, 2], F32)
        nc.sync.dma_start(out=be2p, in_=be2p_d[:, :])
        id64 = singles.tile([128, 64], BF16)
        nc.sync.dma_start(out=id64, in_=id64_d[:, :])
        ones = singles.tile([128, 1], F32)
        nc.sync.dma_start(out=ones, in_=ones_d[:, :])
        zeros = singles.tile([128, GSZ], F32)
        nc.vector.memset(zeros, 0.0)
        eps_pp = singles.tile([128, 1], F32)
        nc.vector.memset(eps_pp, BN_EPS)

        # ---- persistent h1 buffer: [128, C] bf16 ----
        h1 = persist.tile([128, C], BF16)

        # accumulators for BN1 stats
        acc_sum = stats.tile([128, NG], F32)
        acc_sq = stats.tile([128, NG], F32)

        # ================= phase 1: mm1, BN1 stat partials ==============
        with tc.tile_pool(name="xin", bufs=2) as xpool, \
             tc.tile_pool(name="p1ps", bufs=2, space="PSUM") as p1ps, \
             tc.tile_pool(name="trash1", bufs=2) as trashp:
            for g in range(NG):
                c0 = g * GSZ
                xt = xpool.tile([64, GSZ], BF16)
                nc.sync.dma_start(out=xt, in_=xp_d[:, c0:c0 + GSZ])
                ps = p1ps.tile([128, GSZ], F32)
                for j in range(GSZ // 512):
                    nc.tensor.matmul(
                        ps[:, j * 512:(j + 1) * 512],
                        lhsT=w1bd,
                        rhs=xt[:, j * 512:(j + 1) * 512],
                        start=True, stop=True,
                    )
                # h1raw (no bias; cancels through BN) + per-channel sum
                nc.scalar.activation(
                    out=h1[:, c0:c0 + GSZ], in_=ps, func=AF.Copy,
                    bias=0.0, scale=1.0,
                    accum_out=acc_sum[:, g:g + 1],
                )
                tr = trashp.tile([128, GSZ], BF16)
                nc.vector.scalar_tensor_tensor(
                    out=tr,
                    in0=h1[:, c0:c0 + GSZ], scalar=0.0,
                    in1=h1[:, c0:c0 + GSZ],
                    op0=ALU.add, op1=ALU.mult,
                    accum_out=acc_sq[:, g:g + 1],
                )

        if stage < 2:
            return dummy_out()

        # ---- reduce partials, fold halves, AllReduce #1 ----
        packed = stats.tile([128, 2], F32)
        trs = stats.tile([128, NG], F32)
        nc.vector.tensor_scalar(
            out=trs, in0=acc_sum, scalar1=0.0, scalar2=None,
            op0=ALU.add, op1=ALU.add, accum_out=packed[:, 0:1])
        nc.vector.tensor_scalar(
            out=trs, in0=acc_sq, scalar1=0.0, scalar2=None,
            op0=ALU.add, op1=ALU.add, accum_out=packed[:, 1:2])
        fold = stats.tile([64, 2], F32)
        nc.sync.dma_start(out=fold, in_=packed[64:128, :])
        ar_stage = stats.tile([64, 2], F32)
        nc.vector.tensor_add(ar_stage, packed[0:64, :], fold)

        ar1_in = dram.tile([64, 2], F32)
        ar1_out = dram.tile([64, 2], F32)
        nc.sync.dma_start(out=ar1_in, in_=ar_stage)
        nc.gpsimd.collective_compute(
            "AllReduce", ALU.add,
            replica_groups=[list(range(N_CORES))],
            ins=[ar1_in.opt()], outs=[ar1_out.opt()],
        )
        g1stats = stats.tile([128, 2], F32)
        nc.sync.dma_start(out=g1stats[0:64, :], in_=ar1_out)
        nc.sync.dma_start(out=g1stats[64:128, :], in_=ar1_out)

        if stage < 3:
            return dummy_out()

        # ---- BN1 coeffs: a1 = g1*rsqrt(var+eps), c1 = beta1 - a1*mean ----
        meanE = stats.tile([128, 2], F32)
        nc.vector.tensor_scalar_mul(meanE, g1stats, inv_n)
        msq = stats.tile([128, 1], F32)
        nc.vector.tensor_mul(msq, meanE[:, 0:1], meanE[:, 0:1])
        var1 = stats.tile([128, 1], F32)
        nc.vector.tensor_sub(var1, meanE[:, 1:2], msq)
        std1 = stats.tile([128, 1], F32)
        nc.scalar.activation(out=std1, in_=var1, func=AF.Sqrt, bias=eps_pp, scale=1.0)
        rstd1 = stats.tile([128, 1], F32)
        nc.vector.reciprocal(rstd1, std1)
        a1 = stats.tile([128, 1], F32)
        nc.vector.tensor_mul(a1, g1r, rstd1)
        c1t = stats.tile([128, 1], F32)
        nc.vector.tensor_mul(c1t, a1, meanE[:, 0:1])
        c1 = stats.tile([128, 1], F32)
        nc.vector.tensor_sub(c1, be1r, c1t)

        if stage < 4:
            return dummy_out()

        # ================= phase 2: relu in place, Gram(h1) ==============
        s1p = stats.tile([128, NG], F32)
        for g in range(NG):
            c0 = g * GSZ
            nc.scalar.activation(
                out=h1[:, c0:c0 + GSZ], in_=h1[:, c0:c0 + GSZ], func=AF.Relu,
                bias=c1, scale=a1,
                accum_out=s1p[:, g:g + 1],
            )

        gpack = stats.tile([64, 65], F32)
        s1pp = stats.tile([128, 1], F32)
        nc.vector.tensor_scalar(
            out=trs, in0=s1p, scalar1=0.0, scalar2=None,
            op0=ALU.add, op1=ALU.add, accum_out=s1pp)
        s1f = stats.tile([64, 1], F32)
        nc.sync.dma_start(out=s1f, in_=s1pp[64:128, :])
        nc.vector.tensor_add(gpack[:, 64:65], s1pp[0:64, :], s1f)

        if stage < 5:
            return dummy_out()

        # transposes + gram accumulation
        TPG = min(32, n_chunk_t)        # transpose chunks per PSUM group
        n_tg = n_chunk_t // TPG
        with tc.tile_pool(name="tps", bufs=2, space="PSUM") as tpsp, \
             tc.tile_pool(name="g1ps", bufs=1, space="PSUM") as g1psp, \
             tc.tile_pool(name="tsb", bufs=2) as tsbp:
            g1_ps = g1psp.tile([64, 64], F32)
            for tg in range(n_tg):
                tps = tpsp.tile([128, 64 * TPG], BF16)
                for i in range(TPG):
                    chunk = tg * TPG + i
                    hh = chunk // (n_chunk_t // 2)
                    span = (chunk % (n_chunk_t // 2)) * 128
                    nc.tensor.transpose(
                        tps[:, 64 * i:64 * i + 64],
                        in_=h1[64 * hh:64 * hh + 64, span:span + 128],
                        identity=id64[64 * hh:64 * hh + 64, :],
                    )
                tsb = tsbp.tile([128, 64 * TPG], BF16)
                nc.vector.tensor_copy(out=tsb, in_=tps)
                if stage >= 6:
                    for i in range(TPG):
                        chunk = tg * TPG + i
                        nc.tensor.matmul(
                            g1_ps,
                            lhsT=tsb[:, 64 * i:64 * i + 64],
                            rhs=tsb[:, 64 * i:64 * i + 64],
                            start=(chunk == 0), stop=(chunk == n_chunk_t - 1),
                        )
            if stage >= 6:
                nc.vector.tensor_copy(out=gpack[:, 0:64], in_=g1_ps)

        if stage < 7:
            return dummy_out()

        # ---- AllReduce #2 (Gram + s1) ----
        ar2_in = dram.tile([64, 65], F32)
        ar2_out = dram.tile([64, 65], F32)
        nc.sync.dma_start(out=ar2_in, in_=gpack)
        nc.gpsimd.collective_compute(
            "AllReduce", ALU.add,
            replica_groups=[list(range(N_CORES))],
            ins=[ar2_in.opt()], outs=[ar2_out.opt()],
        )
        gsb = stats.tile([64, 65], F32)
        nc.sync.dma_start(out=gsb, in_=ar2_out)

        if stage < 8:
            return dummy_out()

        # ---- BN2 coeffs from Gram ----
        with tc.tile_pool(name="c2ps", bufs=1, space="PSUM") as c2ps:
            t_ps = c2ps.tile([64, 256], F32)
            nc.tensor.matmul(t_ps, lhsT=gsb[:, 0:64], rhs=w2t[0:64, :],
                             start=True, stop=True)
            t_sb = stats.tile([64, 256], F32)
            nc.vector.tensor_copy(out=t_sb, in_=t_ps)
            m_sb = stats.tile([64, 256], F32)
            nc.vector.tensor_mul(m_sb, t_sb, w2t[0:64, :])
            qr_ps = c2ps.tile([128, 4], F32)
            nc.tensor.matmul(qr_ps[:, 0:1], lhsT=m_sb[:, 0:128],
                             rhs=ones[0:64, :], start=True, stop=True)
            nc.tensor.matmul(qr_ps[:, 1:2], lhsT=m_sb[:, 128:256],
                             rhs=ones[0:64, :], start=True, stop=True)
            nc.tensor.matmul(qr_ps[:, 2:3], lhsT=w2t[0:64, 0:128],
                             rhs=gsb[:, 64:65], start=True, stop=True)
            nc.tensor.matmul(qr_ps[:, 3:4], lhsT=w2t[0:64, 128:256],
                             rhs=gsb[:, 64:65], start=True, stop=True)
            qr = stats.tile([128, 4], F32)
            nc.vector.tensor_copy(out=qr, in_=qr_ps)

        qn = stats.tile([128, 2], F32)
        nc.vector.tensor_scalar_mul(qn, qr[:, 0:2], inv_n)
        mr = stats.tile([128, 2], F32)
        nc.vector.tensor_scalar_mul(mr, qr[:, 2:4], inv_n)
        mr2 = stats.tile([128, 2], F32)
        nc.vector.tensor_mul(mr2, mr, mr)
        var2 = stats.tile([128, 2], F32)
        nc.vector.tensor_sub(var2, qn, mr2)
        std2 = stats.tile([128, 2], F32)
        nc.scalar.activation(out=std2, in_=var2, func=AF.Sqrt, bias=eps_pp, scale=1.0)
        rstd2 = stats.tile([128, 2], F32)
        nc.vector.reciprocal(rstd2, std2)
        a2 = stats.tile([128, 2], F32)
        nc.vector.tensor_mul(a2, g2p, rstd2)
        c2t = stats.tile([128, 2], F32)
        nc.vector.tensor_mul(c2t, a2, mr)
        c2 = stats.tile([128, 2], F32)
        nc.vector.tensor_sub(c2, be2p, c2t)

        # a2 broadcast along free axis -> scale W2 columns
        a2d = dram.tile([2, 128], F32)
        nc.sync.dma_start(out=a2d.rearrange("j p -> p j"), in_=a2)
        a2b = stats.tile([128, 256], F32)
        a2b_src = bass.AP(tensor=a2d.tensor, offset=a2d.offset,
                          ap=[[0, 128], [1, 256]])
        nc.sync.dma_start(out=a2b, in_=a2b_src)
        w2a_f = stats.tile([128, 256], F32)
        nc.vector.tensor_mul(w2a_f, w2t, a2b)
        w2a = stats.tile([128, 256], BF16)
        nc.vector.tensor_copy(out=w2a, in_=w2a_f)

        if stage < 9:
            return dummy_out()

        # ================= phase 3: mm2 + relu + segment sums ============
        # segment-aligned spans within a group
        def spans(g):
            res = []
            c0 = g * GSZ
            c1 = c0 + GSZ
            s = c0 // seg_cols
            while c0 < c1:
                e = min(c1, (s + 1) * seg_cols)
                res.append((c0 - g * GSZ, e - c0, s))
                c0 = e
                s += 1
            return res

        nsub = len(spans(0))
        parts0 = stats.tile([128, 2 * NG * nsub], F32)
        parts1 = stats.tile([128, 2 * NG * nsub], F32)
        parts = [parts0, parts1]

        idx = 0
        with tc.tile_pool(name="p3ps", bufs=2, space="PSUM") as p3ps, \
             tc.tile_pool(name="scr3", bufs=3) as scrp:
            for ch in range(2):
                for ph in range(2):
                    for g in range(NG):
                        ps = p3ps.tile([128, GSZ], F32)
                        for j in range(GSZ // 512):
                            c0 = g * GSZ + j * 512
                            nc.tensor.matmul(
                                ps[:, j * 512:(j + 1) * 512],
                                lhsT=w2a[64 * ph:64 * ph + 64,
                                         128 * ch:128 * ch + 128],
                                rhs=h1[64 * ph:64 * ph + 64, c0:c0 + 512],
                                start=True, stop=True,
                            )
                        scr = scrp.tile([128, GSZ], BF16)
                        for si, (off, sz, snum) in enumerate(spans(g)):
                            base = (ph * NG + g) * nsub + si
                            tgt = parts[ch][:, base:base + 1]
                            if idx % 2 == 0:
                                nc.scalar.activation(
                                    out=scr[:, off:off + sz],
                                    in_=ps[:, off:off + sz], func=AF.Relu,
                                    bias=c2[:, ch:ch + 1], scale=1.0,
                                    accum_out=tgt,
                                )
                            else:
                                nc.vector.scalar_tensor_tensor(
                                    out=scr[:, off:off + sz],
                                    in0=ps[:, off:off + sz],
                                    scalar=c2[:, ch:ch + 1],
                                    in1=zeros[:, off:off + sz],
                                    op0=ALU.add, op1=ALU.max,
                                    accum_out=tgt,
                                )
                            idx += 1

        if stage < 10:
            return dummy_out()

        # ---- segment means ----
        gps = NG * nsub // 4  # partial cols per segment (within a half)
        means0 = stats.tile([128, SEG_PER_CORE], F32)
        means1 = stats.tile([128, SEG_PER_CORE], F32)
        means = [means0, means1]
        trg = stats.tile([128, gps], F32)
        for ch in range(2):
            for s in range(SEG_PER_CORE):
                ph = s // 4
                base = ph * NG * nsub + (s % 4) * gps
                nc.vector.tensor_scalar(
                    out=trg, in0=parts[ch][:, base:base + gps],
                    scalar1=0.0, scalar2=None, op0=ALU.add, op1=ALU.add,
                    accum_out=means[ch][:, s:s + 1])
            nc.vector.tensor_scalar_mul(means[ch], means[ch], inv_seg)

        # ---- L2 normalization ----
        with tc.tile_pool(name="l2ps", bufs=1, space="PSUM") as l2ps:
            sq0 = stats.tile([128, SEG_PER_CORE], F32)
            nc.vector.tensor_mul(sq0, means0, means0)
            sq1 = stats.tile([128, SEG_PER_CORE], F32)
            nc.vector.tensor_mul(sq1, means1, means1)
            ns_ps = l2ps.tile([SEG_PER_CORE, 1], F32)
            nc.tensor.matmul(ns_ps, lhsT=sq0, rhs=ones, start=True, stop=False)
            nc.tensor.matmul(ns_ps, lhsT=sq1, rhs=ones, start=False, stop=True)
            nrm = stats.tile([SEG_PER_CORE, 1], F32)
            nc.scalar.activation(out=nrm, in_=ns_ps, func=AF.Sqrt,
                                 bias=zeros[0:SEG_PER_CORE, 0:1], scale=1.0)
        nrmc = stats.tile([SEG_PER_CORE, 1], F32)
        nc.vector.tensor_scalar_max(nrmc, nrm, L2_EPS)
        rin = stats.tile([SEG_PER_CORE, 1], F32)
        nc.vector.reciprocal(rin, nrmc)
        rind = dram.tile([SEG_PER_CORE, 1], F32)
        nc.sync.dma_start(out=rind, in_=rin)
        rb = stats.tile([128, SEG_PER_CORE], F32)
        rb_src = bass.AP(tensor=rind.tensor, offset=rind.offset,
                         ap=[[0, 128], [1, SEG_PER_CORE]])
        nc.sync.dma_start(out=rb, in_=rb_src)

        fin0 = stats.tile([128, SEG_PER_CORE], F32)
        fin1 = stats.tile([128, SEG_PER_CORE], F32)
        out_full = out_d[:, :]
        for ch, fin in ((0, fin0), (1, fin1)):
            nc.vector.tensor_mul(fin, means[ch], rb)
            out_ap = bass.AP(tensor=out_full.tensor,
                             offset=out_full.offset + 128 * ch,
                             ap=[[1, 128], [256, SEG_PER_CORE]])
            nc.sync.dma_start(out=out_ap, in_=fin)

    with tile.TileContext(nc) as tc, contextlib.ExitStack() as ctx:
        _emit(tc, ctx)
    nc.compile()
    return nc


@functools.lru_cache(maxsize=4)
def _get_program(npts, n_total):
    return build_program(npts, n_total)


# --------------------------------------------------------------------------
# host side
# --------------------------------------------------------------------------

def _prep_inputs(x, length, W1, b1, g1, beta1, W2, b2, g2, beta2):
    n = x.shape[0]
    npts = n // N_CORES
    C = npts // 2

    w1bd = np.zeros((64, 128), np.float32)
    w1bd[0:32, 0:64] = np.asarray(W1, np.float32).T
    w1bd[32:64, 64:128] = np.asarray(W1, np.float32).T
    w1bd = w1bd.astype(bf16)

    def rep2(v):  # [64] -> [128,1]
        return np.ascontiguousarray(
            np.tile(np.asarray(v, np.float32), 2)[:, None])

    def pp(v):  # [256] -> [128,2], col j = channel p+128j
        return np.ascontiguousarray(np.asarray(v, np.float32).reshape(2, 128).T)

    common = {
        "w1bd": w1bd,
        "g1r": rep2(g1), "be1r": rep2(beta1),
        "w2t": np.ascontiguousarray(np.vstack([np.asarray(W2, np.float32).T] * 2)),
        "g2p": pp(g2), "be2p": pp(beta2),
        "id64": np.vstack([np.eye(64), np.eye(64)]).astype(bf16),
        "ones128": np.ones((128, 1), np.float32),
    }

    in_maps = []
    for c in range(N_CORES):
        shard = np.asarray(x[c * npts:(c + 1) * npts], np.float32)
        # [npts,32] -> [64, C]: row ch + 32*h holds channel ch of half h
        xp = shard.reshape(2, C, 32).transpose(0, 2, 1)
        in_maps.append({"xp": np.ascontiguousarray(xp).reshape(64, C).astype(bf16),
                        **common})
    return in_maps


def _reference_np(x, length, W1, b1, g1, beta1, W2, b2, g2, beta2):
    """numpy fallback (only used for input shapes this kernel doesn't target)."""
    x = np.asarray(x, np.float64)

    def bn_relu(h, g, be):
        m = h.mean(0)
        v = h.var(0)
        return np.maximum(g * (h - m) / np.sqrt(v + BN_EPS) + be, 0.0)

    h = bn_relu(x @ np.asarray(W1, np.float64).T + b1, g1, beta1)
    h = bn_relu(h @ np.asarray(W2, np.float64).T + b2, g2, beta2)
    length = np.asarray(length)
    sums = np.add.reduceat(h, np.concatenate([[0], np.cumsum(length)[:-1]]), axis=0)
    means = sums / length[:, None].astype(np.float64)
    nrm = np.linalg.norm(means, axis=1, keepdims=True)
    return (means / np.maximum(nrm, L2_EPS)).astype(np.float32)


def kernel(x, length, W1, b1, g1, beta1, W2, b2, g2, beta2):
    length = np.asarray(length)
    n = int(x.shape[0])
    npts = n // N_CORES
    # fast path requires equal-sized segments (what setup_inputs produces)
    if not (np.all(length == length[0]) and n % N_CORES == 0
            and npts % (8 * 512) == 0 and int(length[0]) * SEG_PER_CORE == npts):
        return _reference_np(x, length, W1, b1, g1, beta1, W2, b2, g2, beta2)

    nc = _get_program(npts, n)
    in_maps = _prep_inputs(x, length, W1, b1, g1, beta1, W2, b2, g2, beta2)
    res = run_bass_kernel_spmd(nc, in_maps, core_ids=list(range(N_CORES)))
    return np.concatenate([res.results[c]["out"] for c in range(N_CORES)], axis=0)


# revision 13
# speedup vs baseline: 1.0953x; 1.0953x over previous
"""Trainium2 Bass kernel for nn_FCGF_MLP3 (MLP -> BN -> relu x2 -> segment mean -> L2 norm).

Contract: kernel(**inputs) takes FULL unsharded numpy inputs (as produced by
setup_inputs) and returns the FULL [64, 256] float32 output.  Internally the
points are sharded across 8 NeuronCores (whole segments per core); BN batch
stats are combined with two tiny on-device AllReduces.

Per-core dataflow (npts = 65536 points, channels on partitions):
  phase 1: stream pre-transposed x (bf16), block-diag W1 matmul computes two
           512-pt chunks per matmul -> h1raw [128, npts/2] bf16 resident in
           SBUF; ACT fuses the PSUM->SBUF copy with a per-channel running
           sum; DVE fuses square+sum.  (b1/b2 cancel through BN and are
           dropped entirely.)
  AR1:     AllReduce [64,2] of (sum, sumsq) -> BN1 affine (a1, c1) on device.
  phase 2: ACT in-place relu(a1*h+c1) with free running sum (s1); PE
           transposes 64x128 chunks; PE Gram G1 = h1^T h1 in PSUM.
  AR2:     AllReduce [64,65] of (G1 | s1) -> BN2 stats analytically:
           var2 = q/n - (r/n)^2 with q = diag(W2 G1 W2^T), r = W2 s1.
           a2 folds into W2 columns, c2 stays as per-partition bias.
  phase 3: mm2 (bf16) -> PSUM; fused relu+segment-sum split between ACT
           (activation accum_out) and DVE (scalar_tensor_tensor accum_out);
           segment means; L2 norm via PE column-sum + sqrt + reciprocal.
"""

import contextlib
import functools

import numpy as np
import ml_dtypes

import concourse.bass as bass
import concourse.bacc as bacc
import concourse.tile as tile
from concourse import mybir
from concourse.bass_utils import run_bass_kernel_spmd

BF16 = mybir.dt.bfloat16
F32 = mybir.dt.float32
AF = mybir.ActivationFunctionType
ALU = mybir.AluOpType

N_CORES = 8
N_SEG = 64
SEG_PER_CORE = N_SEG // N_CORES  # 8
BN_EPS = 1e-5
L2_EPS = 1e-12

bf16 = ml_dtypes.bfloat16


# --------------------------------------------------------------------------
# device program
# --------------------------------------------------------------------------

def build_program(npts, n_total, stage=10):
    """Build the per-core bass program.

    Layout: point p of the core lives in column (p mod C) of partition-half
    (p div C), C = npts/2; h1[ch + 64*half, col].  Segments 0..3 of the core
    are in half 0, segments 4..7 in half 1.

    stage < 10 truncates the program after a phase (debug bisection).
    """
    assert npts % (8 * 512) == 0
    C = npts // 2               # columns per half
    seg_cols = npts // 8        # one segment's column span (within one half)
    GSZ = 2048 if C % 2048 == 0 else 512  # columns per PSUM group
    NG = C // GSZ               # groups per half
    n_chunk_t = npts // 128     # number of 128-point transpose chunks

    nc = bacc.Bacc(
        "TRN2",
        target_bir_lowering=False,
        debug=False,
        enable_asserts=True,
        num_devices=N_CORES,
    )

    # ---- I/O ----
    xp_d = nc.dram_tensor("xp", [64, C], BF16, kind="ExternalInput")
    w1bd_d = nc.dram_tensor("w1bd", [64, 128], BF16, kind="ExternalInput")
    g1r_d = nc.dram_tensor("g1r", [128, 1], F32, kind="ExternalInput")
    be1r_d = nc.dram_tensor("be1r", [128, 1], F32, kind="ExternalInput")
    w2t_d = nc.dram_tensor("w2t", [128, 256], F32, kind="ExternalInput")
    g2p_d = nc.dram_tensor("g2p", [128, 2], F32, kind="ExternalInput")
    be2p_d = nc.dram_tensor("be2p", [128, 2], F32, kind="ExternalInput")
    id64_d = nc.dram_tensor("id64", [128, 64], BF16, kind="ExternalInput")
    ones_d = nc.dram_tensor("ones128", [128, 1], F32, kind="ExternalInput")
    out_d = nc.dram_tensor("out", [SEG_PER_CORE, 256], F32, kind="ExternalOutput")

    inv_n = 1.0 / float(n_total)
    inv_seg = 1.0 / float(npts // 8)

    def _emit(tc, ctx):
        singles = ctx.enter_context(tc.tile_pool(name="singles", bufs=1))
        persist = ctx.enter_context(tc.tile_pool(name="persist", bufs=1))
        stats = ctx.enter_context(tc.tile_pool(name="stats", bufs=1))
        dram = ctx.enter_context(tc.tile_pool(name="dram", bufs=1, space="DRAM"))

        def dummy_out():
            dummy = stats.tile([SEG_PER_CORE, 256], F32, name="dummy")
            nc.vector.memset(dummy, 1.0)
            nc.sync.dma_start(out=out_d[:, :], in_=dummy)

        # ---- small constants into SBUF ----
        w1bd = singles.tile([64, 128], BF16)
        nc.sync.dma_start(out=w1bd, in_=w1bd_d[:, :])
        g1r = singles.tile([128, 1], F32)
        nc.sync.dma_start(out=g1r, in_=g1r_d[:, :])
        be1r = singles.tile([128, 1], F32)
        nc.sync.dma_start(out=be1r, in_=be1r_d[:, :])
        w2t = singles.tile([128, 256], F32)
        nc.sync.dma_start(out=w2t, in_=w2t_d[:, :])
        g2p = singles.tile([128, 2], F32)
        nc.sync.dma_start(out=g2p, in_=g2p_d[:, :])
        be2p = singles.tile([128, 2], F32)
        nc.sync.dma_start(out=be2p, in_=be2p_d[:, :])
        id64 = singles.tile([128, 64], BF16)
        nc.sync.dma_start(out=id64, in_=id64_d[:, :])
        ones = singles.tile([128, 1], F32)
        nc.sync.dma_start(out=ones, in_=ones_d[:, :])
        zeros = singles.tile([128, GSZ], F32)
        nc.vector.memset(zeros, 0.0)
        eps_pp = singles.tile([128, 1], F32)
        nc.vector.memset(eps_pp, BN_EPS)

        # ---- persistent h1 buffer: [128, C] bf16 ----
        h1 = persist.tile([128, C], BF16)

        # accumulators for BN1 stats
        acc_sum = stats.tile([128, NG], F32)
        acc_sq = stats.tile([128, NG], F32)

        # PE HAM warm-up machinery: dummy matmul bursts keep the PE clock at
        # 2.4 GHz.  Without them the whole kernel runs at K=4/8 (1.2 GHz).
        wsrc = singles.tile([64, 512], BF16)
        nc.vector.memset(wsrc, 0.0)
        warm_sink = stats.tile([1, 1], F32)
        sink_d = dram.tile([1, 1], F32, name="sink_d")

        def warm_burst(n_mm, dep_src=None, tag="wb"):
            with tc.tile_pool(name=f"wps_{tag}", bufs=1, space="PSUM") as wpsp:
                if dep_src is not None:
                    # tiny copy creates a dependency so the burst is scheduled
                    # after the collective result lands (i.e. during/after the
                    # wait), not at kernel start
                    nc.vector.tensor_copy(out=wsrc[0:64, 0:1], in_=dep_src)
                dps = wpsp.tile([128, 512], F32, name=f"dps_{tag}")
                for _ in range(n_mm):
                    nc.tensor.matmul(dps, lhsT=w1bd, rhs=wsrc,
                                     start=True, stop=True)
                # keep the burst live against any dead-code elimination
                nc.vector.tensor_copy(out=warm_sink, in_=dps[0:1, 0:1])

        warm_burst(24, tag="a")

        # ================= phase 1: mm1, BN1 stat partials ==============
        with tc.tile_pool(name="xin", bufs=2) as xpool, \
             tc.tile_pool(name="p1ps", bufs=2, space="PSUM") as p1ps, \
             tc.tile_pool(name="trash1", bufs=2) as trashp:
            for g in range(NG):
                c0 = g * GSZ
                xt = xpool.tile([64, GSZ], BF16)
                nc.sync.dma_start(out=xt, in_=xp_d[:, c0:c0 + GSZ])
                ps = p1ps.tile([128, GSZ], F32)
                for j in range(GSZ // 512):
                    nc.tensor.matmul(
                        ps[:, j * 512:(j + 1) * 512],
                        lhsT=w1bd,
                        rhs=xt[:, j * 512:(j + 1) * 512],
                        start=True, stop=True,
                    )
                # h1raw (no bias; cancels through BN) + per-channel sum
                nc.scalar.activation(
                    out=h1[:, c0:c0 + GSZ], in_=ps, func=AF.Copy,
                    bias=0.0, scale=1.0,
                    accum_out=acc_sum[:, g:g + 1],
                )
                tr = trashp.tile([128, GSZ], BF16)
                nc.vector.scalar_tensor_tensor(
                    out=tr,
                    in0=h1[:, c0:c0 + GSZ], scalar=0.0,
                    in1=h1[:, c0:c0 + GSZ],
                    op0=ALU.add, op1=ALU.mult,
                    accum_out=acc_sq[:, g:g + 1],
                )

        if stage < 2:
            return dummy_out()

        # ---- reduce partials, fold halves, AllReduce #1 ----
        packed = stats.tile([128, 2], F32)
        trs = stats.tile([128, NG], F32)
        nc.vector.tensor_scalar(
            out=trs, in0=acc_sum, scalar1=0.0, scalar2=None,
            op0=ALU.add, op1=ALU.add, accum_out=packed[:, 0:1])
        nc.vector.tensor_scalar(
            out=trs, in0=acc_sq, scalar1=0.0, scalar2=None,
            op0=ALU.add, op1=ALU.add, accum_out=packed[:, 1:2])
        fold = stats.tile([64, 2], F32)
        nc.sync.dma_start(out=fold, in_=packed[64:128, :])
        ar_stage = stats.tile([64, 2], F32)
        nc.vector.tensor_add(ar_stage, packed[0:64, :], fold)

        ar1_in = dram.tile([64, 2], F32)
        ar1_out = dram.tile([64, 2], F32)
        nc.sync.dma_start(out=ar1_in, in_=ar_stage)
        nc.gpsimd.collective_compute(
            "AllReduce", ALU.add,
            replica_groups=[list(range(N_CORES))],
            ins=[ar1_in.opt()], outs=[ar1_out.opt()],
        )
        g1stats = stats.tile([128, 2], F32)
        nc.sync.dma_start(out=g1stats[0:64, :], in_=ar1_out)
        nc.sync.dma_start(out=g1stats[64:128, :], in_=ar1_out)
        warm_burst(12, dep_src=g1stats[0:64, 0:1], tag="b")

        if stage < 3:
            return dummy_out()

        # ---- BN1 coeffs: a1 = g1*rsqrt(var+eps), c1 = beta1 - a1*mean ----
        meanE = stats.tile([128, 2], F32)
        nc.vector.tensor_scalar_mul(meanE, g1stats, inv_n)
        msq = stats.tile([128, 1], F32)
        nc.vector.tensor_mul(msq, meanE[:, 0:1], meanE[:, 0:1])
        var1 = stats.tile([128, 1], F32)
        nc.vector.tensor_sub(var1, meanE[:, 1:2], msq)
        std1 = stats.tile([128, 1], F32)
        nc.scalar.activation(out=std1, in_=var1, func=AF.Sqrt, bias=eps_pp, scale=1.0)
        rstd1 = stats.tile([128, 1], F32)
        nc.vector.reciprocal(rstd1, std1)
        a1 = stats.tile([128, 1], F32)
        nc.vector.tensor_mul(a1, g1r, rstd1)
        c1t = stats.tile([128, 1], F32)
        nc.vector.tensor_mul(c1t, a1, meanE[:, 0:1])
        c1 = stats.tile([128, 1], F32)
        nc.vector.tensor_sub(c1, be1r, c1t)

        if stage < 4:
            return dummy_out()

        # ================= phase 2: relu in place, Gram(h1) ==============
        s1p = stats.tile([128, NG], F32)
        for g in range(NG):
            c0 = g * GSZ
            nc.scalar.activation(
                out=h1[:, c0:c0 + GSZ], in_=h1[:, c0:c0 + GSZ], func=AF.Relu,
                bias=c1, scale=a1,
                accum_out=s1p[:, g:g + 1],
            )

        gpack = stats.tile([64, 65], F32)
        s1pp = stats.tile([128, 1], F32)
        nc.vector.tensor_scalar(
            out=trs, in0=s1p, scalar1=0.0, scalar2=None,
            op0=ALU.add, op1=ALU.add, accum_out=s1pp)
        s1f = stats.tile([64, 1], F32)
        nc.sync.dma_start(out=s1f, in_=s1pp[64:128, :])
        nc.vector.tensor_add(gpack[:, 64:65], s1pp[0:64, :], s1f)

        if stage < 5:
            return dummy_out()

        # transposes + gram accumulation
        TPG = min(32, n_chunk_t)        # transpose chunks per PSUM group
        n_tg = n_chunk_t // TPG
        with tc.tile_pool(name="tps", bufs=2, space="PSUM") as tpsp, \
             tc.tile_pool(name="g1ps", bufs=1, space="PSUM") as g1psp, \
             tc.tile_pool(name="tsb", bufs=2) as tsbp:
            g1_ps = g1psp.tile([64, 64], F32)
            for tg in range(n_tg):
                tps = tpsp.tile([128, 64 * TPG], BF16)
                for i in range(TPG):
                    chunk = tg * TPG + i
                    hh = chunk // (n_chunk_t // 2)
                    span = (chunk % (n_chunk_t // 2)) * 128
                    nc.tensor.transpose(
                        tps[:, 64 * i:64 * i + 64],
                        in_=h1[64 * hh:64 * hh + 64, span:span + 128],
                        identity=id64[64 * hh:64 * hh + 64, :],
                    )
                tsb = tsbp.tile([128, 64 * TPG], BF16)
                nc.vector.tensor_copy(out=tsb, in_=tps)
                if stage >= 6:
                    for i in range(TPG):
                        chunk = tg * TPG + i
                        nc.tensor.matmul(
                            g1_ps,
                            lhsT=tsb[:, 64 * i:64 * i + 64],
                            rhs=tsb[:, 64 * i:64 * i + 64],
                            start=(chunk == 0), stop=(chunk == n_chunk_t - 1),
                        )
            if stage >= 6:
                nc.vector.tensor_copy(out=gpack[:, 0:64], in_=g1_ps)

        if stage < 7:
            return dummy_out()

        # ---- AllReduce #2 (Gram + s1) ----
        ar2_in = dram.tile([64, 65], F32)
        ar2_out = dram.tile([64, 65], F32)
        nc.sync.dma_start(out=ar2_in, in_=gpack)
        nc.gpsimd.collective_compute(
            "AllReduce", ALU.add,
            replica_groups=[list(range(N_CORES))],
            ins=[ar2_in.opt()], outs=[ar2_out.opt()],
        )
        gsb = stats.tile([64, 65], F32)
        nc.sync.dma_start(out=gsb, in_=ar2_out)
        warm_burst(12, dep_src=gsb[:, 0:1], tag="c")

        if stage < 8:
            return dummy_out()

        # ---- BN2 coeffs from Gram ----
        with tc.tile_pool(name="c2ps", bufs=1, space="PSUM") as c2ps:
            t_ps = c2ps.tile([64, 256], F32)
            nc.tensor.matmul(t_ps, lhsT=gsb[:, 0:64], rhs=w2t[0:64, :],
                             start=True, stop=True)
            t_sb = stats.tile([64, 256], F32)
            nc.vector.tensor_copy(out=t_sb, in_=t_ps)
            m_sb = stats.tile([64, 256], F32)
            nc.vector.tensor_mul(m_sb, t_sb, w2t[0:64, :])
            qr_ps = c2ps.tile([128, 4], F32)
            nc.tensor.matmul(qr_ps[:, 0:1], lhsT=m_sb[:, 0:128],
                             rhs=ones[0:64, :], start=True, stop=True)
            nc.tensor.matmul(qr_ps[:, 1:2], lhsT=m_sb[:, 128:256],
                             rhs=ones[0:64, :], start=True, stop=True)
            nc.tensor.matmul(qr_ps[:, 2:3], lhsT=w2t[0:64, 0:128],
                             rhs=gsb[:, 64:65], start=True, stop=True)
            nc.tensor.matmul(qr_ps[:, 3:4], lhsT=w2t[0:64, 128:256],
                             rhs=gsb[:, 64:65], start=True, stop=True)
            qr = stats.tile([128, 4], F32)
            nc.vector.tensor_copy(out=qr, in_=qr_ps)

        qn = stats.tile([128, 2], F32)
        nc.vector.tensor_scalar_mul(qn, qr[:, 0:2], inv_n)
        mr = stats.tile([128, 2], F32)
        nc.vector.tensor_scalar_mul(mr, qr[:, 2:4], inv_n)
        mr2 = stats.tile([128, 2], F32)
        nc.vector.tensor_mul(mr2, mr, mr)
        var2 = stats.tile([128, 2], F32)
        nc.vector.tensor_sub(var2, qn, mr2)
        std2 = stats.tile([128, 2], F32)
        nc.scalar.activation(out=std2, in_=var2, func=AF.Sqrt, bias=eps_pp, scale=1.0)
        rstd2 = stats.tile([128, 2], F32)
        nc.vector.reciprocal(rstd2, std2)
        a2 = stats.tile([128, 2], F32)
        nc.vector.tensor_mul(a2, g2p, rstd2)
        c2t = stats.tile([128, 2], F32)
        nc.vector.tensor_mul(c2t, a2, mr)
        c2 = stats.tile([128, 2], F32)
        nc.vector.tensor_sub(c2, be2p, c2t)

        # a2 broadcast along free axis -> scale W2 columns
        a2d = dram.tile([2, 128], F32)
        nc.sync.dma_start(out=a2d.rearrange("j p -> p j"), in_=a2)
        a2b = stats.tile([128, 256], F32)
        a2b_src = bass.AP(tensor=a2d.tensor, offset=a2d.offset,
                          ap=[[0, 128], [1, 256]])
        nc.sync.dma_start(out=a2b, in_=a2b_src)
        w2a_f = stats.tile([128, 256], F32)
        nc.vector.tensor_mul(w2a_f, w2t, a2b)
        w2a = stats.tile([128, 256], BF16)
        nc.vector.tensor_copy(out=w2a, in_=w2a_f)

        if stage < 9:
            return dummy_out()

        # ================= phase 3: mm2 + relu + segment sums ============
        # segment-aligned spans within a group
        def spans(g):
            res = []
            c0 = g * GSZ
            c1 = c0 + GSZ
            s = c0 // seg_cols
            while c0 < c1:
                e = min(c1, (s + 1) * seg_cols)
                res.append((c0 - g * GSZ, e - c0, s))
                c0 = e
                s += 1
            return res

        nsub = len(spans(0))
        parts0 = stats.tile([128, 2 * NG * nsub], F32)
        parts1 = stats.tile([128, 2 * NG * nsub], F32)
        parts = [parts0, parts1]

        idx = 0
        with tc.tile_pool(name="p3ps", bufs=2, space="PSUM") as p3ps, \
             tc.tile_pool(name="scr3", bufs=3) as scrp:
            for ch in range(2):
                for ph in range(2):
                    for g in range(NG):
                        ps = p3ps.tile([128, GSZ], F32)
                        for j in range(GSZ // 512):
                            c0 = g * GSZ + j * 512
                            nc.tensor.matmul(
                                ps[:, j * 512:(j + 1) * 512],
                                lhsT=w2a[64 * ph:64 * ph + 64,
                                         128 * ch:128 * ch + 128],
                                rhs=h1[64 * ph:64 * ph + 64, c0:c0 + 512],
                                start=True, stop=True,
                            )
                        scr = scrp.tile([128, GSZ], BF16)
                        for si, (off, sz, snum) in enumerate(spans(g)):
                            base = (ph * NG + g) * nsub + si
                            tgt = parts[ch][:, base:base + 1]
                            if idx % 2 == 0:
                                nc.scalar.activation(
                                    out=scr[:, off:off + sz],
                                    in_=ps[:, off:off + sz], func=AF.Relu,
                                    bias=c2[:, ch:ch + 1], scale=1.0,
                                    accum_out=tgt,
                                )
                            else:
                                nc.vector.scalar_tensor_tensor(
                                    out=scr[:, off:off + sz],
                                    in0=ps[:, off:off + sz],
                                    scalar=c2[:, ch:ch + 1],
                                    in1=zeros[:, off:off + sz],
                                    op0=ALU.add, op1=ALU.max,
                                    accum_out=tgt,
                                )
                            idx += 1

        if stage < 10:
            return dummy_out()

        # ---- segment means ----
        gps = NG * nsub // 4  # partial cols per segment (within a half)
        means0 = stats.tile([128, SEG_PER_CORE], F32)
        means1 = stats.tile([128, SEG_PER_CORE], F32)
        means = [means0, means1]
        trg = stats.tile([128, gps], F32)
        for ch in range(2):
            for s in range(SEG_PER_CORE):
                ph = s // 4
                base = ph * NG * nsub + (s % 4) * gps
                nc.vector.tensor_scalar(
                    out=trg, in0=parts[ch][:, base:base + gps],
                    scalar1=0.0, scalar2=None, op0=ALU.add, op1=ALU.add,
                    accum_out=means[ch][:, s:s + 1])
            nc.vector.tensor_scalar_mul(means[ch], means[ch], inv_seg)

        # ---- L2 normalization ----
        with tc.tile_pool(name="l2ps", bufs=1, space="PSUM") as l2ps:
            sq0 = stats.tile([128, SEG_PER_CORE], F32)
            nc.vector.tensor_mul(sq0, means0, means0)
            sq1 = stats.tile([128, SEG_PER_CORE], F32)
            nc.vector.tensor_mul(sq1, means1, means1)
            ns_ps = l2ps.tile([SEG_PER_CORE, 1], F32)
            nc.tensor.matmul(ns_ps, lhsT=sq0, rhs=ones, start=True, stop=False)
            nc.tensor.matmul(ns_ps, lhsT=sq1, rhs=ones, start=False, stop=True)
            nrm = stats.tile([SEG_PER_CORE, 1], F32)
            nc.scalar.activation(out=nrm, in_=ns_ps, func=AF.Sqrt,
                                 bias=zeros[0:SEG_PER_CORE, 0:1], scale=1.0)
        nrmc = stats.tile([SEG_PER_CORE, 1], F32)
        nc.vector.tensor_scalar_max(nrmc, nrm, L2_EPS)
        rin = stats.tile([SEG_PER_CORE, 1], F32)
        nc.vector.reciprocal(rin, nrmc)
        rind = dram.tile([SEG_PER_CORE, 1], F32)
        nc.sync.dma_start(out=rind, in_=rin)
        rb = stats.tile([128, SEG_PER_CORE], F32)
        rb_src = bass.AP(tensor=rind.tensor, offset=rind.offset,
                         ap=[[0, 128], [1, SEG_PER_CORE]])
        nc.sync.dma_start(out=rb, in_=rb_src)

        fin0 = stats.tile([128, SEG_PER_CORE], F32)
        fin1 = stats.tile([128, SEG_PER_CORE], F32)
        out_full = out_d[:, :]
        for ch, fin in ((0, fin0), (1, fin1)):
            nc.vector.tensor_mul(fin, means[ch], rb)
            out_ap = bass.AP(tensor=out_full.tensor,
                             offset=out_full.offset + 128 * ch,
                             ap=[[1, 128], [256, SEG_PER_CORE]])
            nc.sync.dma_start(out=out_ap, in_=fin)
        nc.sync.dma_start(out=sink_d, in_=warm_sink)

    with tile.TileContext(nc) as tc, contextlib.ExitStack() as ctx:
        _emit(tc, ctx)
    nc.compile()
    return nc


@functools.lru_cache(maxsize=4)
def _get_program(npts, n_total):
    return build_program(npts, n_total)


# --------------------------------------------------------------------------
# host side
# --------------------------------------------------------------------------

def _prep_inputs(x, length, W1, b1, g1, beta1, W2, b2, g2, beta2):
    n = x.shape[0]
    npts = n // N_CORES
    C = npts // 2

    w1bd = np.zeros((64, 128), np.float32)
    w1bd[0:32, 0:64] = np.asarray(W1, np.float32).T
    w1bd[32:64, 64:128] = np.asarray(W1, np.float32).T
    w1bd = w1bd.astype(bf16)

    def rep2(v):  # [64] -> [128,1]
        return np.ascontiguousarray(
            np.tile(np.asarray(v, np.float32), 2)[:, None])

    def pp(v):  # [256] -> [128,2], col j = channel p+128j
        return np.ascontiguousarray(np.asarray(v, np.float32).reshape(2, 128).T)

    common = {
        "w1bd": w1bd,
        "g1r": rep2(g1), "be1r": rep2(beta1),
        "w2t": np.ascontiguousarray(np.vstack([np.asarray(W2, np.float32).T] * 2)),
        "g2p": pp(g2), "be2p": pp(beta2),
        "id64": np.vstack([np.eye(64), np.eye(64)]).astype(bf16),
        "ones128": np.ones((128, 1), np.float32),
    }

    in_maps = []
    for c in range(N_CORES):
        shard = np.asarray(x[c * npts:(c + 1) * npts], np.float32)
        # [npts,32] -> [64, C]: row ch + 32*h holds channel ch of half h
        xp = shard.reshape(2, C, 32).transpose(0, 2, 1)
        in_maps.append({"xp": np.ascontiguousarray(xp).reshape(64, C).astype(bf16),
                        **common})
    return in_maps


def _reference_np(x, length, W1, b1, g1, beta1, W2, b2, g2, beta2):
    """numpy fallback (only used for input shapes this kernel doesn't target)."""
    x = np.asarray(x, np.float64)

    def bn_relu(h, g, be):
        m = h.mean(0)
        v = h.var(0)
        return np.maximum(g * (h - m) / np.sqrt(v + BN_EPS) + be, 0.0)

    h = bn_relu(x @ np.asarray(W1, np.float64).T + b1, g1, beta1)
    h = bn_relu(h @ np.asarray(W2, np.float64).T + b2, g2, beta2)
    length = np.asarray(length)
    sums = np.add.reduceat(h, np.concatenate([[0], np.cumsum(length)[:-1]]), axis=0)
    means = sums / length[:, None].astype(np.float64)
    nrm = np.linalg.norm(means, axis=1, keepdims=True)
    return (means / np.maximum(nrm, L2_EPS)).astype(np.float32)


def kernel(x, length, W1, b1, g1, beta1, W2, b2, g2, beta2):
    length = np.asarray(length)
    n = int(x.shape[0])
    npts = n // N_CORES
    # fast path requires equal-sized segments (what setup_inputs produces)
    if not (np.all(length == length[0]) and n % N_CORES == 0
            and npts % (8 * 512) == 0 and int(length[0]) * SEG_PER_CORE == npts):
        return _reference_np(x, length, W1, b1, g1, beta1, W2, b2, g2, beta2)

    nc = _get_program(npts, n)
    in_maps = _prep_inputs(x, length, W1, b1, g1, beta1, W2, b2, g2, beta2)
    res = run_bass_kernel_spmd(nc, in_maps, core_ids=list(range(N_CORES)))
    return np.concatenate([res.results[c]["out"] for c in range(N_CORES)], axis=0)


# revision 18
# speedup vs baseline: 1.1494x; 1.0494x over previous
"""Trainium2 Bass kernel for nn_FCGF_MLP3 (MLP -> BN -> relu x2 -> segment mean -> L2 norm).

Contract: kernel(**inputs) takes FULL unsharded numpy inputs (as produced by
setup_inputs) and returns the FULL [64, 256] float32 output.  Internally the
points are sharded across 8 NeuronCores (whole segments per core); BN batch
stats are combined with two tiny on-device AllReduces.

Per-core dataflow (npts = 65536 points, channels on partitions):
  phase 1: stream pre-transposed x (bf16), block-diag W1 matmul computes two
           512-pt chunks per matmul -> h1raw [128, npts/2] bf16 resident in
           SBUF; ACT fuses the PSUM->SBUF copy with a per-channel running
           sum; DVE fuses square+sum.  (b1/b2 cancel through BN and are
           dropped entirely.)
  AR1:     AllReduce [64,2] of (sum, sumsq) -> BN1 affine (a1, c1) on device.
  phase 2: ACT in-place relu(a1*h+c1) with free running sum (s1); PE
           transposes 64x128 chunks; PE Gram G1 = h1^T h1 in PSUM.
  AR2:     AllReduce [64,65] of (G1 | s1) -> BN2 stats analytically:
           var2 = q/n - (r/n)^2 with q = diag(W2 G1 W2^T), r = W2 s1.
           a2 folds into W2 columns, c2 stays as per-partition bias.
  phase 3: mm2 (bf16) -> PSUM; fused relu+segment-sum split between ACT
           (activation accum_out) and DVE (scalar_tensor_tensor accum_out);
           segment means; L2 norm via PE column-sum + sqrt + reciprocal.
"""

import contextlib
import functools

import numpy as np
import ml_dtypes

import concourse.bass as bass
import concourse.bacc as bacc
import concourse.tile as tile
from concourse import mybir
from concourse.bass_utils import run_bass_kernel_spmd

BF16 = mybir.dt.bfloat16
F32 = mybir.dt.float32
AF = mybir.ActivationFunctionType
ALU = mybir.AluOpType

N_CORES = 8
N_SEG = 64
SEG_PER_CORE = N_SEG // N_CORES  # 8
BN_EPS = 1e-5
L2_EPS = 1e-12

bf16 = ml_dtypes.bfloat16


# --------------------------------------------------------------------------
# device program
# --------------------------------------------------------------------------

def build_program(npts, n_total, stage=10):
    """Build the per-core bass program.

    Layout: point p of the core lives in column (p mod C) of partition-half
    (p div C), C = npts/2; h1[ch + 64*half, col].  Segments 0..3 of the core
    are in half 0, segments 4..7 in half 1.

    stage < 10 truncates the program after a phase (debug bisection).
    """
    assert npts % (8 * 512) == 0
    C = npts // 2               # columns per half
    seg_cols = npts // 8        # one segment's column span (within one half)
    GSZ = 2048 if C % 2048 == 0 else 512  # columns per PSUM group
    NG = C // GSZ               # groups per half
    n_chunk_t = npts // 128     # number of 128-point transpose chunks

    nc = bacc.Bacc(
        "TRN2",
        target_bir_lowering=False,
        debug=False,
        enable_asserts=True,
        num_devices=N_CORES,
    )

    # ---- I/O ----
    xp_d = nc.dram_tensor("xp", [64, C], BF16, kind="ExternalInput")
    w1bd_d = nc.dram_tensor("w1bd", [64, 128], BF16, kind="ExternalInput")
    g1r_d = nc.dram_tensor("g1r", [128, 1], F32, kind="ExternalInput")
    be1r_d = nc.dram_tensor("be1r", [128, 1], F32, kind="ExternalInput")
    w2t_d = nc.dram_tensor("w2t", [128, 256], F32, kind="ExternalInput")
    g2p_d = nc.dram_tensor("g2p", [128, 2], F32, kind="ExternalInput")
    be2p_d = nc.dram_tensor("be2p", [128, 2], F32, kind="ExternalInput")
    id64_d = nc.dram_tensor("id64", [128, 64], BF16, kind="ExternalInput")
    ones_d = nc.dram_tensor("ones128", [128, 1], F32, kind="ExternalInput")
    out_d = nc.dram_tensor("out", [SEG_PER_CORE, 256], F32, kind="ExternalOutput")

    inv_n = 1.0 / float(n_total)
    inv_seg = 1.0 / float(npts // 8)

    def _emit(tc, ctx):
        singles = ctx.enter_context(tc.tile_pool(name="singles", bufs=1))
        persist = ctx.enter_context(tc.tile_pool(name="persist", bufs=1))
        stats = ctx.enter_context(tc.tile_pool(name="stats", bufs=1))
        dram = ctx.enter_context(tc.tile_pool(name="dram", bufs=1, space="DRAM"))

        def dummy_out():
            dummy = stats.tile([SEG_PER_CORE, 256], F32, name="dummy")
            nc.vector.memset(dummy, 1.0)
            nc.sync.dma_start(out=out_d[:, :], in_=dummy)

        # ---- small constants into SBUF ----
        w1bd = singles.tile([64, 128], BF16)
        nc.sync.dma_start(out=w1bd, in_=w1bd_d[:, :])
        g1r = singles.tile([128, 1], F32)
        nc.sync.dma_start(out=g1r, in_=g1r_d[:, :])
        be1r = singles.tile([128, 1], F32)
        nc.sync.dma_start(out=be1r, in_=be1r_d[:, :])
        w2t = singles.tile([128, 256], F32)
        nc.sync.dma_start(out=w2t, in_=w2t_d[:, :])
        g2p = singles.tile([128, 2], F32)
        nc.sync.dma_start(out=g2p, in_=g2p_d[:, :])
        be2p = singles.tile([128, 2], F32)
        nc.sync.dma_start(out=be2p, in_=be2p_d[:, :])
        id64 = singles.tile([128, 64], BF16)
        nc.sync.dma_start(out=id64, in_=id64_d[:, :])
        ones = singles.tile([128, 1], F32)
        nc.sync.dma_start(out=ones, in_=ones_d[:, :])
        zeros = singles.tile([128, GSZ], F32)
        nc.vector.memset(zeros, 0.0)
        eps_pp = singles.tile([128, 1], F32)
        nc.vector.memset(eps_pp, BN_EPS)

        # ---- persistent h1 buffer: [128, C] bf16 ----
        h1 = persist.tile([128, C], BF16)

        # accumulators for BN1 stats
        acc_sum = stats.tile([128, NG], F32)
        acc_sq = stats.tile([128, NG], F32)

        # ================= phase 1: mm1, BN1 stat partials ==============
        with tc.tile_pool(name="xin", bufs=2) as xpool, \
             tc.tile_pool(name="p1ps", bufs=2, space="PSUM") as p1ps, \
             tc.tile_pool(name="trash1", bufs=2) as trashp:
            for g in range(NG):
                c0 = g * GSZ
                xt = xpool.tile([64, GSZ], BF16)
                nc.sync.dma_start(out=xt, in_=xp_d[:, c0:c0 + GSZ])
                ps = p1ps.tile([128, GSZ], F32)
                MN = 512
                for j in range(GSZ // MN):
                    nc.tensor.matmul(
                        ps[:, j * MN:(j + 1) * MN],
                        lhsT=w1bd,
                        rhs=xt[:, j * MN:(j + 1) * MN],
                        start=True, stop=True,
                    )
                # h1raw (no bias; cancels through BN) + per-channel sum
                nc.scalar.activation(
                    out=h1[:, c0:c0 + GSZ], in_=ps, func=AF.Copy,
                    bias=0.0, scale=1.0,
                    accum_out=acc_sum[:, g:g + 1],
                )
                tr = trashp.tile([128, GSZ], BF16)
                nc.vector.scalar_tensor_tensor(
                    out=tr,
                    in0=h1[:, c0:c0 + GSZ], scalar=0.0,
                    in1=h1[:, c0:c0 + GSZ],
                    op0=ALU.add, op1=ALU.mult,
                    accum_out=acc_sq[:, g:g + 1],
                )

        if stage < 2:
            return dummy_out()

        # ---- reduce partials, fold halves, AllReduce #1 ----
        packed = stats.tile([128, 2], F32)
        trs = stats.tile([128, NG], F32)
        nc.vector.tensor_scalar(
            out=trs, in0=acc_sum, scalar1=0.0, scalar2=None,
            op0=ALU.add, op1=ALU.add, accum_out=packed[:, 0:1])
        nc.vector.tensor_scalar(
            out=trs, in0=acc_sq, scalar1=0.0, scalar2=None,
            op0=ALU.add, op1=ALU.add, accum_out=packed[:, 1:2])
        fold = stats.tile([64, 2], F32)
        nc.sync.dma_start(out=fold, in_=packed[64:128, :])
        ar_stage = stats.tile([64, 2], F32)
        nc.vector.tensor_add(ar_stage, packed[0:64, :], fold)

        ar1_in = dram.tile([64, 2], F32)
        ar1_out = dram.tile([64, 2], F32)
        nc.sync.dma_start(out=ar1_in, in_=ar_stage)
        nc.gpsimd.collective_compute(
            "AllReduce", ALU.add,
            replica_groups=[list(range(N_CORES))],
            ins=[ar1_in.opt()], outs=[ar1_out.opt()],
        )
        g1stats = stats.tile([128, 2], F32)
        nc.sync.dma_start(out=g1stats[0:64, :], in_=ar1_out)
        nc.sync.dma_start(out=g1stats[64:128, :], in_=ar1_out)

        if stage < 3:
            return dummy_out()

        # ---- BN1 coeffs: a1 = g1*rsqrt(var+eps), c1 = beta1 - a1*mean ----
        meanE = stats.tile([128, 2], F32)
        nc.vector.tensor_scalar_mul(meanE, g1stats, inv_n)
        msq = stats.tile([128, 1], F32)
        nc.vector.tensor_mul(msq, meanE[:, 0:1], meanE[:, 0:1])
        var1 = stats.tile([128, 1], F32)
        nc.vector.tensor_sub(var1, meanE[:, 1:2], msq)
        std1 = stats.tile([128, 1], F32)
        nc.scalar.activation(out=std1, in_=var1, func=AF.Sqrt, bias=eps_pp, scale=1.0)
        rstd1 = stats.tile([128, 1], F32)
        nc.vector.reciprocal(rstd1, std1)
        a1 = stats.tile([128, 1], F32)
        nc.vector.tensor_mul(a1, g1r, rstd1)
        c1t = stats.tile([128, 1], F32)
        nc.vector.tensor_mul(c1t, a1, meanE[:, 0:1])
        c1 = stats.tile([128, 1], F32)
        nc.vector.tensor_sub(c1, be1r, c1t)

        if stage < 4:
            return dummy_out()

        # ================= phase 2: relu in place, Gram(h1) ==============
        s1p = stats.tile([128, NG], F32)
        for g in range(NG):
            c0 = g * GSZ
            nc.scalar.activation(
                out=h1[:, c0:c0 + GSZ], in_=h1[:, c0:c0 + GSZ], func=AF.Relu,
                bias=c1, scale=a1,
                accum_out=s1p[:, g:g + 1],
            )

        gpack = stats.tile([64, 65], F32)
        s1pp = stats.tile([128, 1], F32)
        nc.vector.tensor_scalar(
            out=trs, in0=s1p, scalar1=0.0, scalar2=None,
            op0=ALU.add, op1=ALU.add, accum_out=s1pp)
        s1f = stats.tile([64, 1], F32)
        nc.sync.dma_start(out=s1f, in_=s1pp[64:128, :])
        nc.vector.tensor_add(gpack[:, 64:65], s1pp[0:64, :], s1f)

        if stage < 5:
            return dummy_out()

        # transposes + gram accumulation.  Chunks from partition-half 0 and
        # half 1 are interleaved so consecutive PE transposes use disjoint
        # row groups (hardware-concurrent); gram pairs go to two column-tiled
        # accumulators (array cols 0:63 / 64:127, also concurrent).
        TPG = min(32, n_chunk_t)        # transpose chunks per PSUM group
        n_tg = n_chunk_t // TPG
        nhalf = n_chunk_t // 2
        with tc.tile_pool(name="tps", bufs=2, space="PSUM") as tpsp, \
             tc.tile_pool(name="g1ps", bufs=1, space="PSUM") as g1psp, \
             tc.tile_pool(name="tsb", bufs=2) as tsbp:
            g1_ps = g1psp.tile([128, 64], F32)
            g1_ps_o = g1psp.tile([128, 64], F32, name="g1_ps_o", tag="g1o")
            for tg in range(n_tg):
                tps = tpsp.tile([128, 64 * TPG], BF16)
                for i in range(TPG):
                    k = tg * TPG + i
                    hh = k // nhalf
                    span = (k % nhalf) * 128
                    nc.tensor.transpose(
                        tps[:, 64 * i:64 * i + 64],
                        in_=h1[64 * hh:64 * hh + 64, span:span + 128],
                        identity=id64[64 * hh:64 * hh + 64, :],
                    )
                tsb = tsbp.tile([128, 64 * TPG], BF16)
                nc.vector.tensor_copy(out=tsb, in_=tps)
                if stage >= 6:
                    for i in range(TPG):
                        k = tg * TPG + i
                        par = k % 2       # even -> array cols 0:63, odd -> 64:127
                        dst = g1_ps[0:64, :] if par == 0 else g1_ps_o[64:128, :]
                        nc.tensor.matmul(
                            dst,
                            lhsT=tsb[:, 64 * i:64 * i + 64],
                            rhs=tsb[:, 64 * i:64 * i + 64],
                            start=(k < 2), stop=(k >= n_chunk_t - 2),
                        )
            if stage >= 6:
                gtmp = stats.tile([128, 64], F32, name="gtmp")
                nc.vector.tensor_copy(out=gtmp[0:64, :], in_=g1_ps[0:64, :])
                nc.vector.tensor_copy(out=gtmp[64:128, :], in_=g1_ps_o[64:128, :])
                gfold = stats.tile([64, 64], F32, name="gfold")
                nc.sync.dma_start(out=gfold, in_=gtmp[64:128, :])
                nc.vector.tensor_add(gpack[:, 0:64], gtmp[0:64, :], gfold)

        if stage < 7:
            return dummy_out()

        # ---- AllReduce #2 (Gram + s1) ----
        ar2_in = dram.tile([64, 65], F32)
        ar2_out = dram.tile([64, 65], F32)
        nc.sync.dma_start(out=ar2_in, in_=gpack)
        nc.gpsimd.collective_compute(
            "AllReduce", ALU.add,
            replica_groups=[list(range(N_CORES))],
            ins=[ar2_in.opt()], outs=[ar2_out.opt()],
        )
        gsb = stats.tile([64, 65], F32)
        nc.sync.dma_start(out=gsb, in_=ar2_out)

        if stage < 8:
            return dummy_out()

        # ---- BN2 coeffs from Gram ----
        with tc.tile_pool(name="c2ps", bufs=1, space="PSUM") as c2ps:
            t_ps = c2ps.tile([64, 256], F32)
            nc.tensor.matmul(t_ps, lhsT=gsb[:, 0:64], rhs=w2t[0:64, :],
                             start=True, stop=True)
            t_sb = stats.tile([64, 256], F32)
            nc.vector.tensor_copy(out=t_sb, in_=t_ps)
            m_sb = stats.tile([64, 256], F32)
            nc.vector.tensor_mul(m_sb, t_sb, w2t[0:64, :])
            qr_ps = c2ps.tile([128, 4], F32)
            nc.tensor.matmul(qr_ps[:, 0:1], lhsT=m_sb[:, 0:128],
                             rhs=ones[0:64, :], start=True, stop=True)
            nc.tensor.matmul(qr_ps[:, 1:2], lhsT=m_sb[:, 128:256],
                             rhs=ones[0:64, :], start=True, stop=True)
            nc.tensor.matmul(qr_ps[:, 2:3], lhsT=w2t[0:64, 0:128],
                             rhs=gsb[:, 64:65], start=True, stop=True)
            nc.tensor.matmul(qr_ps[:, 3:4], lhsT=w2t[0:64, 128:256],
                             rhs=gsb[:, 64:65], start=True, stop=True)
            qr = stats.tile([128, 4], F32)
            nc.vector.tensor_copy(out=qr, in_=qr_ps)

        qn = stats.tile([128, 2], F32)
        nc.vector.tensor_scalar_mul(qn, qr[:, 0:2], inv_n)
        mr = stats.tile([128, 2], F32)
        nc.vector.tensor_scalar_mul(mr, qr[:, 2:4], inv_n)
        mr2 = stats.tile([128, 2], F32)
        nc.vector.tensor_mul(mr2, mr, mr)
        var2 = stats.tile([128, 2], F32)
        nc.vector.tensor_sub(var2, qn, mr2)
        std2 = stats.tile([128, 2], F32)
        nc.scalar.activation(out=std2, in_=var2, func=AF.Sqrt, bias=eps_pp, scale=1.0)
        rstd2 = stats.tile([128, 2], F32)
        nc.vector.reciprocal(rstd2, std2)
        a2 = stats.tile([128, 2], F32)
        nc.vector.tensor_mul(a2, g2p, rstd2)
        c2t = stats.tile([128, 2], F32)
        nc.vector.tensor_mul(c2t, a2, mr)
        c2 = stats.tile([128, 2], F32)
        nc.vector.tensor_sub(c2, be2p, c2t)

        # a2 broadcast along free axis -> scale W2 columns
        a2d = dram.tile([2, 128], F32)
        nc.sync.dma_start(out=a2d.rearrange("j p -> p j"), in_=a2)
        a2b = stats.tile([128, 256], F32)
        a2b_src = bass.AP(tensor=a2d.tensor, offset=a2d.offset,
                          ap=[[0, 128], [1, 256]])
        nc.sync.dma_start(out=a2b, in_=a2b_src)
        w2a_f = stats.tile([128, 256], F32)
        nc.vector.tensor_mul(w2a_f, w2t, a2b)
        w2a = stats.tile([128, 256], BF16)
        nc.vector.tensor_copy(out=w2a, in_=w2a_f)

        if stage < 9:
            return dummy_out()

        # ================= phase 3: mm2 + relu + segment sums ============
        # segment-aligned spans within a group
        def spans(g):
            res = []
            c0 = g * GSZ
            c1 = c0 + GSZ
            s = c0 // seg_cols
            while c0 < c1:
                e = min(c1, (s + 1) * seg_cols)
                res.append((c0 - g * GSZ, e - c0, s))
                c0 = e
                s += 1
            return res

        nsub = len(spans(0))
        parts0 = stats.tile([128, 2 * NG * nsub], F32)
        parts1 = stats.tile([128, 2 * NG * nsub], F32)
        parts = [parts0, parts1]

        idx = 0
        with tc.tile_pool(name="p3ps", bufs=2, space="PSUM") as p3ps, \
             tc.tile_pool(name="scr3", bufs=3) as scrp:
            for ch in range(2):
                for ph in range(2):
                    for g in range(NG):
                        ps = p3ps.tile([128, GSZ], F32)
                        MN = 512
                        for j in range(GSZ // MN):
                            c0 = g * GSZ + j * MN
                            nc.tensor.matmul(
                                ps[:, j * MN:(j + 1) * MN],
                                lhsT=w2a[64 * ph:64 * ph + 64,
                                         128 * ch:128 * ch + 128],
                                rhs=h1[64 * ph:64 * ph + 64, c0:c0 + MN],
                                start=True, stop=True,
                            )
                        scr = scrp.tile([128, GSZ], BF16)
                        for si, (off, sz, snum) in enumerate(spans(g)):
                            base = (ph * NG + g) * nsub + si
                            tgt = parts[ch][:, base:base + 1]
                            if idx % 2 == 0:
                                nc.scalar.activation(
                                    out=scr[:, off:off + sz],
                                    in_=ps[:, off:off + sz], func=AF.Relu,
                                    bias=c2[:, ch:ch + 1], scale=1.0,
                                    accum_out=tgt,
                                )
                            else:
                                nc.vector.scalar_tensor_tensor(
                                    out=scr[:, off:off + sz],
                                    in0=ps[:, off:off + sz],
                                    scalar=c2[:, ch:ch + 1],
                                    in1=zeros[:, off:off + sz],
                                    op0=ALU.add, op1=ALU.max,
                                    accum_out=tgt,
                                )
                            idx += 1

        if stage < 10:
            return dummy_out()

        # ---- segment means ----
        gps = NG * nsub // 4  # partial cols per segment (within a half)
        means0 = stats.tile([128, SEG_PER_CORE], F32)
        means1 = stats.tile([128, SEG_PER_CORE], F32)
        means = [means0, means1]
        trg = stats.tile([128, gps], F32)
        for ch in range(2):
            for s in range(SEG_PER_CORE):
                ph = s // 4
                base = ph * NG * nsub + (s % 4) * gps
                nc.vector.tensor_scalar(
                    out=trg, in0=parts[ch][:, base:base + gps],
                    scalar1=0.0, scalar2=None, op0=ALU.add, op1=ALU.add,
                    accum_out=means[ch][:, s:s + 1])
            nc.vector.tensor_scalar_mul(means[ch], means[ch], inv_seg)

        # ---- L2 normalization ----
        with tc.tile_pool(name="l2ps", bufs=1, space="PSUM") as l2ps:
            sq0 = stats.tile([128, SEG_PER_CORE], F32)
            nc.vector.tensor_mul(sq0, means0, means0)
            sq1 = stats.tile([128, SEG_PER_CORE], F32)
            nc.vector.tensor_mul(sq1, means1, means1)
            ns_ps = l2ps.tile([SEG_PER_CORE, 1], F32)
            nc.tensor.matmul(ns_ps, lhsT=sq0, rhs=ones, start=True, stop=False)
            nc.tensor.matmul(ns_ps, lhsT=sq1, rhs=ones, start=False, stop=True)
            nrm = stats.tile([SEG_PER_CORE, 1], F32)
            nc.scalar.activation(out=nrm, in_=ns_ps, func=AF.Sqrt,
                                 bias=zeros[0:SEG_PER_CORE, 0:1], scale=1.0)
        nrmc = stats.tile([SEG_PER_CORE, 1], F32)
        nc.vector.tensor_scalar_max(nrmc, nrm, L2_EPS)
        rin = stats.tile([SEG_PER_CORE, 1], F32)
        nc.vector.reciprocal(rin, nrmc)
        rind = dram.tile([SEG_PER_CORE, 1], F32)
        nc.sync.dma_start(out=rind, in_=rin)
        rb = stats.tile([128, SEG_PER_CORE], F32)
        rb_src = bass.AP(tensor=rind.tensor, offset=rind.offset,
                         ap=[[0, 128], [1, SEG_PER_CORE]])
        nc.sync.dma_start(out=rb, in_=rb_src)

        fin0 = stats.tile([128, SEG_PER_CORE], F32)
        fin1 = stats.tile([128, SEG_PER_CORE], F32)
        out_full = out_d[:, :]
        for ch, fin in ((0, fin0), (1, fin1)):
            nc.vector.tensor_mul(fin, means[ch], rb)
            out_ap = bass.AP(tensor=out_full.tensor,
                             offset=out_full.offset + 128 * ch,
                             ap=[[1, 128], [256, SEG_PER_CORE]])
            nc.sync.dma_start(out=out_ap, in_=fin)

    with tile.TileContext(nc) as tc, contextlib.ExitStack() as ctx:
        _emit(tc, ctx)
    nc.compile()
    return nc


@functools.lru_cache(maxsize=4)
def _get_program(npts, n_total):
    return build_program(npts, n_total)


# --------------------------------------------------------------------------
# host side
# --------------------------------------------------------------------------

def _prep_inputs(x, length, W1, b1, g1, beta1, W2, b2, g2, beta2):
    n = x.shape[0]
    npts = n // N_CORES
    C = npts // 2

    w1bd = np.zeros((64, 128), np.float32)
    w1bd[0:32, 0:64] = np.asarray(W1, np.float32).T
    w1bd[32:64, 64:128] = np.asarray(W1, np.float32).T
    w1bd = w1bd.astype(bf16)

    def rep2(v):  # [64] -> [128,1]
        return np.ascontiguousarray(
            np.tile(np.asarray(v, np.float32), 2)[:, None])

    def pp(v):  # [256] -> [128,2], col j = channel p+128j
        return np.ascontiguousarray(np.asarray(v, np.float32).reshape(2, 128).T)

    common = {
        "w1bd": w1bd,
        "g1r": rep2(g1), "be1r": rep2(beta1),
        "w2t": np.ascontiguousarray(np.vstack([np.asarray(W2, np.float32).T] * 2)),
        "g2p": pp(g2), "be2p": pp(beta2),
        "id64": np.vstack([np.eye(64), np.eye(64)]).astype(bf16),
        "ones128": np.ones((128, 1), np.float32),
    }

    in_maps = []
    for c in range(N_CORES):
        shard = np.asarray(x[c * npts:(c + 1) * npts], np.float32)
        # [npts,32] -> [64, C]: row ch + 32*h holds channel ch of half h
        xp = shard.reshape(2, C, 32).transpose(0, 2, 1)
        in_maps.append({"xp": np.ascontiguousarray(xp).reshape(64, C).astype(bf16),
                        **common})
    return in_maps


def _reference_np(x, length, W1, b1, g1, beta1, W2, b2, g2, beta2):
    """numpy fallback (only used for input shapes this kernel doesn't target)."""
    x = np.asarray(x, np.float64)

    def bn_relu(h, g, be):
        m = h.mean(0)
        v = h.var(0)
        return np.maximum(g * (h - m) / np.sqrt(v + BN_EPS) + be, 0.0)

    h = bn_relu(x @ np.asarray(W1, np.float64).T + b1, g1, beta1)
    h = bn_relu(h @ np.asarray(W2, np.float64).T + b2, g2, beta2)
    length = np.asarray(length)
    sums = np.add.reduceat(h, np.concatenate([[0], np.cumsum(length)[:-1]]), axis=0)
    means = sums / length[:, None].astype(np.float64)
    nrm = np.linalg.norm(means, axis=1, keepdims=True)
    return (means / np.maximum(nrm, L2_EPS)).astype(np.float32)


def kernel(x, length, W1, b1, g1, beta1, W2, b2, g2, beta2):
    length = np.asarray(length)
    n = int(x.shape[0])
    npts = n // N_CORES
    # fast path requires equal-sized segments (what setup_inputs produces)
    if not (np.all(length == length[0]) and n % N_CORES == 0
            and npts % (8 * 512) == 0 and int(length[0]) * SEG_PER_CORE == npts):
        return _reference_np(x, length, W1, b1, g1, beta1, W2, b2, g2, beta2)

    nc = _get_program(npts, n)
    in_maps = _prep_inputs(x, length, W1, b1, g1, beta1, W2, b2, g2, beta2)
    res = run_bass_kernel_spmd(nc, in_maps, core_ids=list(range(N_CORES)))
    return np.concatenate([res.results[c]["out"] for c in range(N_CORES)], axis=0)
